# revision 6
# baseline (speedup 1.0000x reference)
"""Trainium2 Bass kernel for nn_Attention (buggy-reshape attention), 8-core SPMD.

Math (reference): q/k/v = (x @ W).reshape entangles batch and head. Each of the
256 (h,b) "chunks" is a contiguous 64-row block of the projected (16384, 512)
matrices:
  K_c = XK[64c:64c+64, :]            (64=A, 512=M)  -- used as-is
  Q_c = XQ[64c:64c+64, :].reshape(512, 64)
  V_c = XV[64c:64c+64, :].reshape(512, 64)
  out_c = softmax(Q_c @ K_c, -1) @ V_c ; final[b] = relu(mean_h out_(h,b) + x_b @ Wr)

Chunk (h,b) touches only x[4h + b//8, 64*(b%8):64*(b%8)+64, :]. We shard by
OUTPUT batch: core d owns batches 4d..4d+3 (all 8 heads) and is handed exactly
the x rows it needs -> zero collectives; head-mean is local.

Per-core layouts (m-permutation p = 64*s + r where m = 8*r + s; same perm used
for the n axis via host-permuted Wk columns):
  S^T tiles (n'-part, p-free) = Ksb_slice.T @ QTall_slice ; softmax over n'
  (partition axis; no max subtraction -- scores are O(+-50), exp fits fp32
  easily); column sums via ones-matmul; O^T = V_perm.T @ expS, normalized by
  reciprocal broadcast; 1/8 head-mean folded into Wv.
"""

import os
import sys

import numpy as np

sys.path.insert(0, "/opt/trn_rl_repo")

import concourse.bass as bass
import concourse.bacc as bacc
import concourse.mybir as mybir
from concourse.tile import TileContext

FP = mybir.dt.float32
FR = mybir.dt.float32r
AF = mybir.ActivationFunctionType
ALU = mybir.AluOpType

B, M, E, H, A = 32, 512, 256, 8, 64
NCORES = 8

# m (and n) permutation: p = 64*s + r  <->  m = 8*r + s
_M_OF_P = np.array([8 * (p % 64) + p // 64 for p in range(512)])
_P_OF_M = np.array([64 * (m % 8) + m // 8 for m in range(512)])

# dtype knobs for matmul classes (float32 = exact/slow, float32r = fast/reduced)
DT_PROJ = mybir.dt.float32   # Q/K projections (feed exp -> precision-critical)
DT_V = mybir.dt.float32      # V projection
DT_S = mybir.dt.float32      # S^T = K.T @ Q matmul
DT_SUM = mybir.dt.float32    # ones-matmul column sums
DT_O = mybir.dt.float32      # O^T = V.T @ expS
DT_R = mybir.dt.float32      # Wr projection

USE_SHIFT_COPY = True        # engine copy into sbuf partitions 64..127


def _mm(nc, out, lhsT, rhs, dt, start, stop):
    nc.tensor.matmul(out, lhsT, rhs, start=start, stop=stop)


def build_core_graph():
    nc = bacc.Bacc(target_bir_lowering=False)

    xaT_e = nc.declare_dram_parameter("xaT", [E, 2048], FR, isOutput=False)
    xoT_e = nc.declare_dram_parameter("xoT", [E, 2048], FR, isOutput=False)
    wq_e = nc.declare_dram_parameter("wq", [E, 512], FR, isOutput=False)
    wkp_e = nc.declare_dram_parameter("wkp", [E, 512], FR, isOutput=False)
    wv8_e = nc.declare_dram_parameter("wv8", [E, 512], FR, isOutput=False)
    wr_e = nc.declare_dram_parameter("wr", [E, A], FR, isOutput=False)
    ones_e = nc.declare_dram_parameter("ones", [128, A], FR, isOutput=False)
    out_e = nc.declare_dram_parameter("out", [A, 2048], FP, isOutput=True)

    with TileContext(nc) as tc:
        from contextlib import ExitStack

        with ExitStack() as ctx:
            const = ctx.enter_context(tc.tile_pool(name="const", bufs=1))
            qt_pool = ctx.enter_context(tc.tile_pool(name="qt", bufs=2))
            ksb_pool = ctx.enter_context(tc.tile_pool(name="ksb", bufs=16))
            vsb_pool = ctx.enter_context(tc.tile_pool(name="vsb", bufs=16))
            exps_pool = ctx.enter_context(tc.tile_pool(name="exps", bufs=8))
            misc_pool = ctx.enter_context(tc.tile_pool(name="misc", bufs=3))
            acc_pool = ctx.enter_context(tc.tile_pool(name="acc", bufs=4))

            pp_psum = ctx.enter_context(tc.tile_pool(name="pp", bufs=2, space="PSUM"))
            st_psum = ctx.enter_context(tc.tile_pool(name="st", bufs=2, space="PSUM"))
            sb_psum = ctx.enter_context(tc.tile_pool(name="sb", bufs=2, space="PSUM"))
            ot_psum = ctx.enter_context(tc.tile_pool(name="ot", bufs=2, space="PSUM"))

            # ---- load inputs ----
            xaT = [const.tile([128, 2048], FR, tag=f"xaT{k}", name=f"xaT{k}") for k in range(2)]
            xoT = [const.tile([128, 2048], FR, tag=f"xoT{k}", name=f"xoT{k}") for k in range(2)]
            wq = [const.tile([128, 512], FR, tag=f"wq{k}", name=f"wq{k}") for k in range(2)]
            wkp = [const.tile([128, 512], FR, tag=f"wkp{k}", name=f"wkp{k}") for k in range(2)]
            wv8 = [const.tile([128, 512], FR, tag=f"wv8{k}", name=f"wv8{k}") for k in range(2)]
            wr = [const.tile([128, A], FR, tag=f"wr{k}", name=f"wr{k}") for k in range(2)]
            for k in range(2):
                nc.sync.dma_start(out=xaT[k][:], in_=xaT_e[128 * k:128 * k + 128, :])
                nc.sync.dma_start(out=xoT[k][:], in_=xoT_e[128 * k:128 * k + 128, :])
                nc.sync.dma_start(out=wq[k][:], in_=wq_e[128 * k:128 * k + 128, :])
                nc.sync.dma_start(out=wkp[k][:], in_=wkp_e[128 * k:128 * k + 128, :])
                nc.sync.dma_start(out=wv8[k][:], in_=wv8_e[128 * k:128 * k + 128, :])
                nc.sync.dma_start(out=wr[k][:], in_=wr_e[128 * k:128 * k + 128, :])

            ones = const.tile([128, A], FR, tag="ones")
            nc.sync.dma_start(out=ones[:], in_=ones_e[:, :])

            # ---- Wr projection: PRT[a, 512*i + p] ----
            prt = const.tile([A, 4, 512], FP, tag="prt")
            for i in range(4):
                rp = pp_psum.tile([A, 512], FP, tag="pp")
                for k in range(2):
                    _mm(nc, rp[:], wr[k][:], xoT[k][:, 512 * i:512 * i + 512],
                        DT_R, start=(k == 0), stop=(k == 1))
                nc.scalar.copy(prt[:, i, :], rp[:])

            acc = [acc_pool.tile([A, 512], FP, tag="acc", name="acc") for _ in range(4)]

            # ---- main loop: 4 groups of 8 chunks ----
            for g in range(4):
                # Q^T projection for the whole group, per s-slice
                qtall = qt_pool.tile([A, 8, 8, A], FR, tag="qt")
                for s in range(8):
                    qp = pp_psum.tile([A, 512], FP, tag="pp")
                    for k in range(2):
                        _mm(nc, qp[:], wq[k][:, 64 * s:64 * s + 64],
                            xaT[k][:, 512 * g:512 * g + 512],
                            DT_PROJ, start=(k == 0), stop=(k == 1))
                    nc.scalar.copy(qtall[:, s, :, :], qp[:])

                ksb = {}
                vsb = {}
                for jj in range(8):
                    j = 8 * g + jj
                    # K projection (n'-permuted columns via host-permuted Wk)
                    kp = pp_psum.tile([A, 512], FP, tag="pp")
                    for k in range(2):
                        _mm(nc, kp[:], xaT[k][:, 64 * j:64 * j + 64], wkp[k][:],
                            DT_PROJ, start=(k == 0), stop=(k == 1))
                    ksb[jj] = ksb_pool.tile([A, 512], FR, tag="ksb", name="ksb")
                    nc.vector.tensor_copy(ksb[jj][:], kp[:])

                    # V projection -> V_perm tiles (128 n'-part, 4 kn, 64 a)
                    pv = pp_psum.tile([A, 4, 2, A], FP, tag="pp")
                    for k in range(2):
                        _mm(nc, pv[:, :, :, :], xaT[k][:, 64 * j:64 * j + 64],
                            wv8[k][:], DT_V, start=(k == 0), stop=(k == 1))
                    vsb[jj] = vsb_pool.tile([128, 4, A], FR, tag="vsb", name="vsb")
                    nc.scalar.copy(vsb[jj][0:64, :, :], pv[:, :, 0, :])
                    vtmp = misc_pool.tile([A, 4, A], FR, tag="vtmp", name="vtmp")
                    nc.vector.tensor_copy(vtmp[:], pv[:, :, 1, :])
                    nc.sync.dma_start(out=vsb[jj][64:128, :, :], in_=vtmp[:])

                for jj in range(8):
                    j = 8 * g + jj
                    h, i = j // 4, j % 4
                    # S^T tiles + exp
                    es = [exps_pool.tile([128, 512], FR, tag="exps", name="es") for _ in range(4)]
                    for kn in range(4):
                        st = st_psum.tile([128, 512], FP, tag="st")
                        _mm(nc, st[:], ksb[jj][:, 128 * kn:128 * kn + 128],
                            qtall[:, :, jj, :], DT_S, start=True, stop=True)
                        nc.scalar.activation(es[kn][:], st[:], AF.Exp)
                    # column sums (over n' partitions) via ones-matmul
                    sumb = sb_psum.tile([A, 512], FP, tag="sb")
                    for kn in range(4):
                        _mm(nc, sumb[:], ones[:, 0:A], es[kn][:], DT_SUM,
                            start=(kn == 0), stop=(kn == 3))
                    # O^T accumulation
                    ot = ot_psum.tile([A, 512], FP, tag="ot")
                    for kn in range(4):
                        _mm(nc, ot[:], vsb[jj][:, kn, :], es[kn][:], DT_O,
                            start=(kn == 0), stop=(kn == 3))
                    # normalize + head-accumulate
                    recipb = misc_pool.tile([A, 512], FP, tag="recip")
                    nc.vector.reciprocal(recipb[:], sumb[:])
                    if h == 0:
                        nc.vector.tensor_mul(acc[i][:], ot[:], recipb[:])
                    else:
                        otmp = misc_pool.tile([A, 512], FP, tag="otmp")
                        nc.vector.tensor_mul(otmp[:], ot[:], recipb[:])
                        nc.vector.tensor_add(acc[i][:], acc[i][:], otmp[:])

            # ---- epilogue ----
            for i in range(4):
                pre = misc_pool.tile([A, 512], FP, tag="pre")
                nc.vector.tensor_add(pre[:], acc[i][:], prt[:, i, :])
                outsb = misc_pool.tile([A, 512], FP, tag="outsb")
                nc.scalar.activation(outsb[:], pre[:], AF.Relu)
                nc.sync.dma_start(out=out_e[:, 512 * i:512 * i + 512], in_=outsb[:])

    nc.finalize()
    return nc


def _stage_inputs(x, Wq, Wk, Wv, Wr):
    """Build per-core input dicts."""
    Wk_perm = np.ascontiguousarray(Wk[:, _M_OF_P])
    Wv8 = np.ascontiguousarray(Wv / 8.0)
    Wq_c = np.ascontiguousarray(Wq)
    Wr_c = np.ascontiguousarray(Wr)
    in_maps = []
    for d in range(NCORES):
        xa = np.concatenate(
            [x[4 * h + d // 2, 256 * (d % 2):256 * (d % 2) + 256, :] for h in range(H)],
            axis=0)
        xaT = np.ascontiguousarray(xa.T)
        xoT = np.ascontiguousarray(
            np.concatenate([x[4 * d + i][_M_OF_P, :].T for i in range(4)], axis=1))
        in_maps.append({
            "xaT": xaT, "xoT": xoT, "wq": Wq_c, "wkp": Wk_perm,
            "wv8": Wv8, "wr": Wr_c, "ones": np.ones((128, 64), np.float32),
        })
    return in_maps


_CACHED = {}


def kernel(x, Wq, Wk, Wv, Wr, _want_trace=False):
    from concourse.bass_utils import run_bass_kernel_spmd

    x = np.asarray(x, dtype=np.float32)
    in_maps = _stage_inputs(x, np.asarray(Wq, np.float32), np.asarray(Wk, np.float32),
                            np.asarray(Wv, np.float32), np.asarray(Wr, np.float32))

    if "nc" not in _CACHED:
        _CACHED["nc"] = build_core_graph()
    nc = _CACHED["nc"]

    res = run_bass_kernel_spmd(nc, in_maps, core_ids=list(range(NCORES)),
                               trace=_want_trace)
    _CACHED["last_result"] = res

    out = np.zeros((B, M, A), np.float32)
    for d in range(NCORES):
        o = res.results[d]["out"]  # (64, 2048) = (a, 512*i + p)
        for i in range(4):
            out[4 * d + i] = o[:, 512 * i + _P_OF_M].T
    return out


if __name__ == "__main__":
    np.random.seed(0)
    pass


# revision 7
# speedup vs baseline: 1.4036x; 1.4036x over previous
"""Trainium2 Bass kernel for nn_Attention (buggy-reshape attention), 8-core SPMD.

Math (reference): q/k/v = (x @ W).reshape entangles batch and head. Each of the
256 (h,b) "chunks" is a contiguous 64-row block of the projected (16384, 512)
matrices:
  K_c = XK[64c:64c+64, :]            (64=A, 512=M)  -- used as-is
  Q_c = XQ[64c:64c+64, :].reshape(512, 64)
  V_c = XV[64c:64c+64, :].reshape(512, 64)
  out_c = softmax(Q_c @ K_c, -1) @ V_c ; final[b] = relu(mean_h out_(h,b) + x_b @ Wr)

Chunk (h,b) touches only x[4h + b//8, 64*(b%8):64*(b%8)+64, :]. We shard by
OUTPUT batch: core d owns batches 4d..4d+3 (all 8 heads) and is handed exactly
the x rows it needs -> zero collectives; head-mean is local.

Per-core layouts (m-permutation p = 64*s + r where m = 8*r + s; same perm used
for the n axis via host-permuted Wk columns):
  S^T tiles (n'-part, p-free) = Ksb_slice.T @ QTall_slice ; softmax over n'
  (partition axis; no max subtraction -- scores are O(+-50), exp fits fp32
  easily); column sums via ones-matmul; O^T = V_perm.T @ expS, normalized by
  reciprocal broadcast; 1/8 head-mean folded into Wv.
"""

import os
import sys

import numpy as np

sys.path.insert(0, "/opt/trn_rl_repo")

import concourse.bass as bass
import concourse.bacc as bacc
import concourse.mybir as mybir
from concourse.tile import TileContext

FP = mybir.dt.float32
FR = mybir.dt.float32r
AF = mybir.ActivationFunctionType
ALU = mybir.AluOpType

B, M, E, H, A = 32, 512, 256, 8, 64
NCORES = 8

# m (and n) permutation: p = 64*s + r  <->  m = 8*r + s
_M_OF_P = np.array([8 * (p % 64) + p // 64 for p in range(512)])
_P_OF_M = np.array([64 * (m % 8) + m // 8 for m in range(512)])

# dtype knobs for matmul classes (float32 = exact/slow, float32r = fast/reduced)
DT_PROJ = mybir.dt.float32   # Q/K projections (feed exp -> precision-critical)
DT_V = mybir.dt.float32      # V projection
DT_S = mybir.dt.float32      # S^T = K.T @ Q matmul
DT_SUM = mybir.dt.float32    # ones-matmul column sums
DT_O = mybir.dt.float32      # O^T = V.T @ expS
DT_R = mybir.dt.float32      # Wr projection

USE_SHIFT_COPY = True        # engine copy into sbuf partitions 64..127


def _mm(nc, out, lhsT, rhs, dt, start, stop):
    nc.tensor.matmul(out, lhsT, rhs, start=start, stop=stop)


def build_core_graph():
    nc = bacc.Bacc(target_bir_lowering=False)

    xaT_e = nc.declare_dram_parameter("xaT", [E, 2048], FR, isOutput=False)
    xoT_e = nc.declare_dram_parameter("xoT", [E, 2048], FR, isOutput=False)
    wq_e = nc.declare_dram_parameter("wq", [E, 512], FR, isOutput=False)
    wkp_e = nc.declare_dram_parameter("wkp", [E, 512], FR, isOutput=False)
    wv8_e = nc.declare_dram_parameter("wv8", [E, 512], FR, isOutput=False)
    wr_e = nc.declare_dram_parameter("wr", [E, A], FR, isOutput=False)
    ones_e = nc.declare_dram_parameter("ones", [128, A], FR, isOutput=False)
    out_e = nc.declare_dram_parameter("out", [A, 2048], FP, isOutput=True)

    with TileContext(nc) as tc:
        from contextlib import ExitStack

        with ExitStack() as ctx:
            const = ctx.enter_context(tc.tile_pool(name="const", bufs=1))
            qt_pool = ctx.enter_context(tc.tile_pool(name="qt", bufs=2))
            ksb_pool = ctx.enter_context(tc.tile_pool(name="ksb", bufs=16))
            vsb_pool = ctx.enter_context(tc.tile_pool(name="vsb", bufs=16))
            exps_pool = ctx.enter_context(tc.tile_pool(name="exps", bufs=8))
            misc_pool = ctx.enter_context(tc.tile_pool(name="misc", bufs=3))
            acc_pool = ctx.enter_context(tc.tile_pool(name="acc", bufs=4))

            pp_psum = ctx.enter_context(tc.tile_pool(name="pp", bufs=2, space="PSUM"))
            st_psum = ctx.enter_context(tc.tile_pool(name="st", bufs=2, space="PSUM"))
            sb_psum = ctx.enter_context(tc.tile_pool(name="sb", bufs=1, space="PSUM"))
            ot_psum = ctx.enter_context(tc.tile_pool(name="ot", bufs=1, space="PSUM"))

            # ---- load inputs ----
            xaT = [const.tile([128, 2048], FR, tag=f"xaT{k}", name=f"xaT{k}") for k in range(2)]
            xoT = [const.tile([128, 2048], FR, tag=f"xoT{k}", name=f"xoT{k}") for k in range(2)]
            wq = [const.tile([128, 512], FR, tag=f"wq{k}", name=f"wq{k}") for k in range(2)]
            wkp = [const.tile([128, 512], FR, tag=f"wkp{k}", name=f"wkp{k}") for k in range(2)]
            wv8 = [const.tile([128, 512], FR, tag=f"wv8{k}", name=f"wv8{k}") for k in range(2)]
            wr = [const.tile([128, A], FR, tag=f"wr{k}", name=f"wr{k}") for k in range(2)]
            for k in range(2):
                nc.sync.dma_start(out=xaT[k][:], in_=xaT_e[128 * k:128 * k + 128, :])
                nc.sync.dma_start(out=xoT[k][:], in_=xoT_e[128 * k:128 * k + 128, :])
                nc.sync.dma_start(out=wq[k][:], in_=wq_e[128 * k:128 * k + 128, :])
                nc.sync.dma_start(out=wkp[k][:], in_=wkp_e[128 * k:128 * k + 128, :])
                nc.sync.dma_start(out=wv8[k][:], in_=wv8_e[128 * k:128 * k + 128, :])
                nc.sync.dma_start(out=wr[k][:], in_=wr_e[128 * k:128 * k + 128, :])

            ones = const.tile([128, A], FR, tag="ones")
            nc.sync.dma_start(out=ones[:], in_=ones_e[:, :])

            # ---- Wr projection: PRT[a, 512*i + p] ----
            prt = const.tile([A, 4, 512], FP, tag="prt")
            for i in range(4):
                rp = pp_psum.tile([A, 512], FP, tag="pp")
                for k in range(2):
                    _mm(nc, rp[:], wr[k][:], xoT[k][:, 512 * i:512 * i + 512],
                        DT_R, start=(k == 0), stop=(k == 1))
                nc.scalar.copy(prt[:, i, :], rp[:])

            acc = [acc_pool.tile([A, 512], FP, tag="acc", name="acc") for _ in range(4)]

            # ---- main loop: 4 groups of 8 chunks ----
            for g in range(4):
                # Q^T projection for the whole group, per s-slice
                qtall = qt_pool.tile([A, 8, 8, A], FR, tag="qt")
                for s in range(8):
                    qp = pp_psum.tile([A, 512], FP, tag="pp")
                    for k in range(2):
                        _mm(nc, qp[:], wq[k][:, 64 * s:64 * s + 64],
                            xaT[k][:, 512 * g:512 * g + 512],
                            DT_PROJ, start=(k == 0), stop=(k == 1))
                    nc.scalar.copy(qtall[:, s, :, :], qp[:])

                ksb = {}
                vsb = {}
                for jj in range(8):
                    j = 8 * g + jj
                    # K projection (n'-permuted columns via host-permuted Wk)
                    kp = pp_psum.tile([A, 512], FP, tag="pp")
                    for k in range(2):
                        _mm(nc, kp[:], xaT[k][:, 64 * j:64 * j + 64], wkp[k][:],
                            DT_PROJ, start=(k == 0), stop=(k == 1))
                    ksb[jj] = ksb_pool.tile([A, 512], FR, tag="ksb", name="ksb")
                    nc.vector.tensor_copy(ksb[jj][:], kp[:])

                    # V projection -> V_perm tiles (128 n'-part, 4 kn, 64 a)
                    pv = pp_psum.tile([A, 4, 2, A], FP, tag="pp")
                    for k in range(2):
                        _mm(nc, pv[:, :, :, :], xaT[k][:, 64 * j:64 * j + 64],
                            wv8[k][:], DT_V, start=(k == 0), stop=(k == 1))
                    vsb[jj] = vsb_pool.tile([128, 4, A], FR, tag="vsb", name="vsb")
                    nc.scalar.copy(vsb[jj][0:64, :, :], pv[:, :, 0, :])
                    vtmp = misc_pool.tile([A, 4, A], FR, tag="vtmp", name="vtmp")
                    nc.vector.tensor_copy(vtmp[:], pv[:, :, 1, :])
                    nc.sync.dma_start(out=vsb[jj][64:128, :, :], in_=vtmp[:])

                for jj in range(8):
                    j = 8 * g + jj
                    h, i = j // 4, j % 4
                    # S^T tiles + exp
                    es = [exps_pool.tile([128, 2, 512], FR, tag="exps", name="es") for _ in range(2)]
                    for half in range(2):
                        st = st_psum.tile([128, 2, 512], FP, tag="st")
                        for q in range(2):
                            kn = 2 * half + q
                            _mm(nc, st[:, q, :], ksb[jj][:, 128 * kn:128 * kn + 128],
                                qtall[:, :, jj, :], DT_S, start=True, stop=True)
                        nc.scalar.activation(es[half][:], st[:], AF.Exp)
                    # column sums (over n' partitions) via ones-matmul
                    sumb = sb_psum.tile([A, 512], FP, tag="sb")
                    for kn in range(4):
                        _mm(nc, sumb[:], ones[:, 0:A], es[kn // 2][:, kn % 2, :], DT_SUM,
                            start=(kn == 0), stop=(kn == 3))
                    # O^T accumulation
                    ot = ot_psum.tile([A, 512], FP, tag="ot")
                    for kn in range(4):
                        _mm(nc, ot[:], vsb[jj][:, kn, :], es[kn // 2][:, kn % 2, :], DT_O,
                            start=(kn == 0), stop=(kn == 3))
                    # normalize + head-accumulate
                    recipb = misc_pool.tile([A, 512], FP, tag="recip")
                    nc.vector.reciprocal_approx_fast(out=recipb[:], in_=sumb[:])
                    if h == 0:
                        nc.vector.tensor_mul(acc[i][:], ot[:], recipb[:])
                    else:
                        otmp = misc_pool.tile([A, 512], FP, tag="otmp")
                        nc.vector.tensor_mul(otmp[:], ot[:], recipb[:])
                        nc.vector.tensor_add(acc[i][:], acc[i][:], otmp[:])

            # ---- epilogue ----
            for i in range(4):
                pre = misc_pool.tile([A, 512], FP, tag="pre")
                nc.vector.tensor_add(pre[:], acc[i][:], prt[:, i, :])
                outsb = misc_pool.tile([A, 512], FP, tag="outsb")
                nc.scalar.activation(outsb[:], pre[:], AF.Relu)
                nc.sync.dma_start(out=out_e[:, 512 * i:512 * i + 512], in_=outsb[:])

    nc.finalize()
    return nc


def _stage_inputs(x, Wq, Wk, Wv, Wr):
    """Build per-core input dicts."""
    Wk_perm = np.ascontiguousarray(Wk[:, _M_OF_P])
    Wv8 = np.ascontiguousarray(Wv / 8.0)
    Wq_c = np.ascontiguousarray(Wq)
    Wr_c = np.ascontiguousarray(Wr)
    in_maps = []
    for d in range(NCORES):
        xa = np.concatenate(
            [x[4 * h + d // 2, 256 * (d % 2):256 * (d % 2) + 256, :] for h in range(H)],
            axis=0)
        xaT = np.ascontiguousarray(xa.T)
        xoT = np.ascontiguousarray(
            np.concatenate([x[4 * d + i][_M_OF_P, :].T for i in range(4)], axis=1))
        in_maps.append({
            "xaT": xaT, "xoT": xoT, "wq": Wq_c, "wkp": Wk_perm,
            "wv8": Wv8, "wr": Wr_c, "ones": np.ones((128, 64), np.float32),
        })
    return in_maps


_CACHED = {}


def kernel(x, Wq, Wk, Wv, Wr, _want_trace=False):
    from concourse.bass_utils import run_bass_kernel_spmd

    x = np.asarray(x, dtype=np.float32)
    in_maps = _stage_inputs(x, np.asarray(Wq, np.float32), np.asarray(Wk, np.float32),
                            np.asarray(Wv, np.float32), np.asarray(Wr, np.float32))

    if "nc" not in _CACHED:
        _CACHED["nc"] = build_core_graph()
    nc = _CACHED["nc"]

    res = run_bass_kernel_spmd(nc, in_maps, core_ids=list(range(NCORES)),
                               trace=_want_trace)
    _CACHED["last_result"] = res

    out = np.zeros((B, M, A), np.float32)
    for d in range(NCORES):
        o = res.results[d]["out"]  # (64, 2048) = (a, 512*i + p)
        for i in range(4):
            out[4 * d + i] = o[:, 512 * i + _P_OF_M].T
    return out


if __name__ == "__main__":
    np.random.seed(0)
    pass


# revision 13
# speedup vs baseline: 1.9694x; 1.4031x over previous
"""Trainium2 Bass kernel for nn_Attention (buggy-reshape attention), 8-core SPMD.

Math (reference): q/k/v = (x @ W).reshape entangles batch and head. Each of the
256 (h,b) "chunks" is a contiguous 64-row block of the projected (16384, 512)
matrices:
  K_c = XK[64c:64c+64, :]            (64=A, 512=M)  -- used as-is
  Q_c = XQ[64c:64c+64, :].reshape(512, 64)
  V_c = XV[64c:64c+64, :].reshape(512, 64)
  out_c = softmax(Q_c @ K_c, -1) @ V_c ; final[b] = relu(mean_h out_(h,b) + x_b @ Wr)

Chunk (h,b) touches only x[4h + b//8, 64*(b%8):64*(b%8)+64, :]. We shard by
OUTPUT batch: core d owns batches 4d..4d+3 (all 8 heads) and is handed exactly
the x rows it needs -> zero collectives; head-mean is local.

Per-core layouts (m-permutation p = 64*s + r where m = 8*r + s; same perm used
for the n axis via host-permuted Wk columns):
  S^T tiles (n'-part, p-free) = Ksb_slice.T @ QTall_slice ; softmax over n'
  (partition axis; no max subtraction -- scores are O(+-50), exp fits fp32
  easily); column sums via ones-matmul; O^T = V_perm.T @ expS, normalized by
  reciprocal broadcast; 1/8 head-mean folded into Wv.
"""

import os
import sys

import numpy as np

sys.path.insert(0, "/opt/trn_rl_repo")

import concourse.bass as bass
import concourse.bacc as bacc
import concourse.mybir as mybir
from concourse.tile import TileContext

FP = mybir.dt.float32
FR = mybir.dt.float32r
BF = mybir.dt.bfloat16
F16 = mybir.dt.float16
AF = mybir.ActivationFunctionType
ALU = mybir.AluOpType

B, M, E, H, A = 32, 512, 256, 8, 64
NCORES = 8

# m (and n) permutation: p = 64*s + r  <->  m = 8*r + s
_M_OF_P = np.array([8 * (p % 64) + p // 64 for p in range(512)])
_P_OF_M = np.array([64 * (m % 8) + m // 8 for m in range(512)])

# dtype knobs for matmul classes (float32 = exact/slow, float32r = fast/reduced)
DT_PROJ = mybir.dt.float32   # Q/K projections (feed exp -> precision-critical)
DT_V = mybir.dt.float32      # V projection
DT_S = mybir.dt.float32      # S^T = K.T @ Q matmul
DT_SUM = mybir.dt.float32    # ones-matmul column sums
DT_O = mybir.dt.float32      # O^T = V.T @ expS
DT_R = mybir.dt.float32      # Wr projection

USE_SHIFT_COPY = True        # engine copy into sbuf partitions 64..127


def _mm(nc, out, lhsT, rhs, dt, start, stop):
    nc.tensor.matmul(out, lhsT, rhs, start=start, stop=stop)


def build_core_graph():
    nc = bacc.Bacc(target_bir_lowering=False)

    xaT_e = nc.declare_dram_parameter("xaT", [E, 2048], F16, isOutput=False)
    xoT_e = nc.declare_dram_parameter("xoT", [E, 2048], BF, isOutput=False)
    wqk_e = nc.declare_dram_parameter("wqk", [E, 1024], F16, isOutput=False)
    wvr_e = nc.declare_dram_parameter("wvr", [E, 512 + A], BF, isOutput=False)
    ones_e = nc.declare_dram_parameter("ones", [128, A], BF, isOutput=False)
    out_e = nc.declare_dram_parameter("out", [A, 2048], FP, isOutput=True)

    with TileContext(nc) as tc:
        from contextlib import ExitStack

        with ExitStack() as ctx:
            const = ctx.enter_context(tc.tile_pool(name="const", bufs=1))
            qt_pool = ctx.enter_context(tc.tile_pool(name="qt", bufs=2))
            ksb_pool = ctx.enter_context(tc.tile_pool(name="ksb", bufs=8))
            vsb_pool = ctx.enter_context(tc.tile_pool(name="vsb", bufs=16))
            exps_pool = ctx.enter_context(tc.tile_pool(name="exps", bufs=8))
            misc_pool = ctx.enter_context(tc.tile_pool(name="misc", bufs=4))
            acc_pool = ctx.enter_context(tc.tile_pool(name="acc", bufs=4))

            pp_psum = ctx.enter_context(tc.tile_pool(name="pp", bufs=2, space="PSUM"))
            st_psum = ctx.enter_context(tc.tile_pool(name="st", bufs=2, space="PSUM"))
            sb_psum = ctx.enter_context(tc.tile_pool(name="sb", bufs=1, space="PSUM"))
            ot_psum = ctx.enter_context(tc.tile_pool(name="ot", bufs=1, space="PSUM"))

            # ---- load inputs ----
            xaT = [const.tile([128, 2048], FR, tag=f"xaT{k}", name=f"xaT{k}") for k in range(2)]
            xoT = [const.tile([128, 2048], BF, tag=f"xoT{k}", name=f"xoT{k}") for k in range(2)]
            wq = [const.tile([128, 512], F16, tag=f"wq{k}", name=f"wq{k}") for k in range(2)]
            wkp = [const.tile([128, 512], F16, tag=f"wkp{k}", name=f"wkp{k}") for k in range(2)]
            wv8 = [const.tile([128, 512], BF, tag=f"wv8{k}", name=f"wv8{k}") for k in range(2)]
            wr = [const.tile([128, A], BF, tag=f"wr{k}", name=f"wr{k}") for k in range(2)]
            for k in range(2):
                nc.sync.dma_start(out=wq[k][:], in_=wq_e[128 * k:128 * k + 128, :])
                nc.sync.dma_start(out=wkp[k], in_=wkp_e[128 * k:128 * k + 128, :])
                nc.sync.dma_start(out=xaT[k][:], in_=xaT_e[128 * k:128 * k + 128, :])
                nc.sync.dma_start(out=wv8[k], in_=wv8_e[128 * k:128 * k + 128, :])
                nc.sync.dma_start(out=wr[k], in_=wr_e[128 * k:128 * k + 128, :])
                nc.sync.dma_start(out=xoT[k][:], in_=xoT_e[128 * k:128 * k + 128, :])

            ones = const.tile([128, A], BF, tag="ones")
            nc.sync.dma_start(out=ones[:], in_=ones_e[:, :])
            xaTb = [const.tile([128, 2048], BF, tag=f"xaTb{k}", name=f"xaTb{k}") for k in range(2)]
            for k in range(2):
                nc.vector.tensor_copy(xaTb[k][:], xaT[k][:])

            # ---- Wr projection: PRT[a, 512*i + p] ----
            prt = const.tile([A, 4, 512], FP, tag="prt")
            for i in range(4):
                rp = pp_psum.tile([A, 512], FP, tag="pp")
                for k in range(2):
                    _mm(nc, rp[:], wr[k], xoT[k][:, 512 * i:512 * i + 512],
                        DT_R, start=(k == 0), stop=(k == 1))
                nc.scalar.copy(prt[:, i, :], rp[:])

            acc = [acc_pool.tile([A, 512], FP, tag="acc", name="acc") for _ in range(4)]

            # ---- main loop: 4 groups of 8 chunks ----
            for g in range(4):
                # Q^T projection for the whole group, per s-slice
                qtall = qt_pool.tile([A, 8, 8, A], FR, tag="qt")
                for s in range(8):
                    qp = pp_psum.tile([A, 512], FP, tag="pp")
                    for k in range(2):
                        _mm(nc, qp[:], wq[k][:, 64 * s:64 * s + 64],
                            xaT[k][:, 512 * g:512 * g + 512],
                            DT_PROJ, start=(k == 0), stop=(k == 1))
                    nc.scalar.copy(qtall[:, s, :, :], qp[:])

                ksb = {}
                vsb = {}
                for jj in range(8):
                    j = 8 * g + jj
                    # K projection (n'-permuted columns via host-permuted Wk)
                    kp = pp_psum.tile([A, 512], FP, tag="pp")
                    for k in range(2):
                        _mm(nc, kp[:], xaT[k][:, 64 * j:64 * j + 64], wkp[k],
                            DT_PROJ, start=(k == 0), stop=(k == 1))
                    ksb[jj] = ksb_pool.tile([A, 512], FR, tag="ksb", name="ksb")
                    nc.vector.tensor_copy(ksb[jj][:], kp[:])

                    # V projection -> V_perm tiles (128 n'-part, 4 kn, 64 a)
                    pv = pp_psum.tile([A, 4, 2, A], FP, tag="pp")
                    for k in range(2):
                        _mm(nc, pv[:, :, :, :], xaTb[k][:, 64 * j:64 * j + 64],
                            wv8[k], DT_V, start=(k == 0), stop=(k == 1))
                    vsb[jj] = vsb_pool.tile([128, 4, A], BF, tag="vsb", name="vsb")
                    nc.scalar.copy(vsb[jj][0:64, :, :], pv[:, :, 0, :])
                    vtmp = misc_pool.tile([A, 4, A], BF, tag="vtmp", name="vtmp")
                    nc.vector.tensor_copy(vtmp[:], pv[:, :, 1, :])
                    nc.sync.dma_start(out=vsb[jj][64:128, :, :], in_=vtmp[:])

                for jj in range(8):
                    j = 8 * g + jj
                    h, i = j // 4, j % 4
                    # S^T tiles + exp
                    es = [exps_pool.tile([128, 2, 512], BF, tag="exps", name="es") for _ in range(2)]
                    for half in range(2):
                        st = st_psum.tile([128, 2, 512], FP, tag="st")
                        for q in range(2):
                            kn = 2 * half + q
                            _mm(nc, st[:, q, :], ksb[jj][:, 128 * kn:128 * kn + 128],
                                qtall[:, :, jj, :], DT_S, start=True, stop=True)
                        nc.scalar.activation(es[half][:], st[:], AF.Exp)
                    # column sums (over n' partitions) via ones-matmul
                    sumb = sb_psum.tile([A, 512], FP, tag="sb")
                    for kn in range(4):
                        _mm(nc, sumb[:], ones[:, 0:A], es[kn // 2][:, kn % 2, :], DT_SUM,
                            start=(kn == 0), stop=(kn == 3))
                    # O^T accumulation
                    ot = ot_psum.tile([A, 512], FP, tag="ot")
                    for kn in range(4):
                        _mm(nc, ot[:], vsb[jj][:, kn, :], es[kn // 2][:, kn % 2, :], DT_O,
                            start=(kn == 0), stop=(kn == 3))
                    # normalize + head-accumulate
                    recipb = misc_pool.tile([A, 512], FP, tag="recip")
                    nc.vector.reciprocal_approx_fast(out=recipb[:], in_=sumb[:])
                    if h == 0:
                        nc.vector.tensor_mul(acc[i][:], ot[:], recipb[:])
                    else:
                        otmp = misc_pool.tile([A, 512], FP, tag="otmp")
                        nc.vector.tensor_mul(otmp[:], ot[:], recipb[:])
                        nc.vector.tensor_add(acc[i][:], acc[i][:], otmp[:])

            # ---- epilogue ----
            for i in range(4):
                pre = misc_pool.tile([A, 512], FP, tag="pre")
                nc.vector.tensor_add(pre[:], acc[i][:], prt[:, i, :])
                outsb = misc_pool.tile([A, 512], FP, tag="outsb")
                nc.scalar.activation(outsb[:], pre[:], AF.Relu)
                nc.sync.dma_start(out=out_e[:, 512 * i:512 * i + 512], in_=outsb[:])

    nc.finalize()
    return nc


def _stage_inputs(x, Wq, Wk, Wv, Wr):
    """Build per-core input dicts."""
    Wk_perm = np.ascontiguousarray(Wk[:, _M_OF_P].astype(np.float16))
    import ml_dtypes
    BF_NP = ml_dtypes.bfloat16
    Wv8 = np.ascontiguousarray((Wv / 8.0).astype(BF_NP))
    Wq_c = np.ascontiguousarray(Wq.astype(np.float16))
    Wr_c = np.ascontiguousarray(Wr.astype(BF_NP))
    in_maps = []
    for d in range(NCORES):
        xa = np.concatenate(
            [x[4 * h + d // 2, 256 * (d % 2):256 * (d % 2) + 256, :] for h in range(H)],
            axis=0)
        xaT = np.ascontiguousarray(xa.T.astype(np.float16))
        xoT = np.ascontiguousarray(
            np.concatenate([x[4 * d + i][_M_OF_P, :].T for i in range(4)],
                           axis=1).astype(BF_NP))
        in_maps.append({
            "xaT": xaT, "xoT": xoT,
            "wqk": np.concatenate([Wq_c, Wk_perm], axis=1),
            "wvr": np.concatenate([Wv8, Wr_c], axis=1),
            "ones": np.ones((128, 64), BF_NP),
        })
    return in_maps


_CACHED = {}


def kernel(x, Wq, Wk, Wv, Wr, _want_trace=False):
    from concourse.bass_utils import run_bass_kernel_spmd

    x = np.asarray(x, dtype=np.float32)
    in_maps = _stage_inputs(x, np.asarray(Wq, np.float32), np.asarray(Wk, np.float32),
                            np.asarray(Wv, np.float32), np.asarray(Wr, np.float32))

    if "nc" not in _CACHED:
        _CACHED["nc"] = build_core_graph()
    nc = _CACHED["nc"]

    res = run_bass_kernel_spmd(nc, in_maps, core_ids=list(range(NCORES)),
                               trace=_want_trace)
    _CACHED["last_result"] = res

    out = np.zeros((B, M, A), np.float32)
    for d in range(NCORES):
        o = res.results[d]["out"]  # (64, 2048) = (a, 512*i + p)
        for i in range(4):
            out[4 * d + i] = o[:, 512 * i + _P_OF_M].T
    return out


if __name__ == "__main__":
    np.random.seed(0)
    pass
